# revision 44
# baseline (speedup 1.0000x reference)
"""TRN2 Bass kernel for nn_Brain: delayed-synapse recurrent network.

Strategy (banded delay-batched futures, v4):
  total_input[t] = c0 + sum_{d=1}^{15} W_d @ acts_{t-d};  acts_t = tanh(...)
- Edges with delay >= 16 never fire; delay-0 edges give a per-neuron
  constant c0 (host-computed).
- Neurons are SPATIALLY SORTED: delays are int(Euclidean distance) of
  points in a cube, so graph distances (Dijkstra with edge length d+0.5)
  recover a 1D landmark projection; sorting by it makes every W_d BANDED.
  Out-of-band 128x128 weight tiles are skipped.
- Targets sharded contiguously in sorted order (512/core). SPMD identical
  program: per-core band windows become uniform by keeping the activation
  history in CORE-ROTATED chunk coordinates (rotated chunk c holds global
  chunk (4k + c) mod 32 on core k; own chunks always at slots 32..36 of
  the doubled space). The per-step AllGather lands via DynSlice
  (partition_id-offset) DMA windows from a 3x-copied DRAM bounce buffer.
- Latency structure (measured): DMA-completion semaphores post 1.4-4us
  after the wire drains (worse under concurrent DMA); engine-to-engine
  event hops are ~0.1us; the 8-core AllGather mesh is ~8.3us
  trigger-to-done and inflates under concurrent DMA traffic; the FIRST
  collective pays a ~75-95us fabric warm-up that every core idles
  through. Hence:
  * acts_1 = tanh(c0) is host-computed, so step-1's cin is a DRAM-DRAM
    copy with no on-device deps: trigger_1 fires ~10us in and the
    warm-up clock starts immediately.
  * ~15MB of the 22.2MB SBUF-resident banded weights preload during
    the warm-up window; the rest trickles in capped sync/scalar chunks
    (never gpsimd: a chunk there would sit between the c2 copy and the
    next cin in queue order and delay the trigger).
  * Per step, only the d=1 fresh app gates tanh_{t+1}. Its halo slots
    land DIRECTLY from the AllGather output (cmid) via a ring-wrapped
    DynSlice window (edge cores read a wrong-rank block that hits
    all-zero weight tiles). The full 3x-bounce landing is deferred
    until after the NEXT step's cin+trigger, off the critical chain.
  * d>=2 apps use nb<=d-1 windows so they are never fresh; the ones
    whose deadline gates tanh_{t+1} are emitted first and run on the
    PE during the AllGather.
  * cin DMA + trigger + own-acts copy share the GpSimd queue.
- Bucket d applies in windows of nb steps (both batch rows ride as
  matmul columns).
"""
import numpy as np

N_NEURONS = 4096
INPUT_SIZE = 1024
BATCH = 2
STEPS = 16
N_CORES = 8
TGT_PER_CORE = N_NEURONS // N_CORES        # 512
TCH = TGT_PER_CORE // 128                  # 4 target chunks per core
SCH = N_NEURONS // 128                     # 32 global source chunks
MAXD = STEPS - 1                           # delays 1..15 useful
FP8_SCALE = 64.0
MAXB = 8
OWN0, OWN1 = SCH, SCH + TCH                # own chunks: rotated slots 32..36
# weights loaded lazily during steps; the rest preload during the ~80us
# first-collective fabric warm-up that every core sits through
LAZY_D = (6, 10, 11, 15)

_compiled = None
_compiled_key = None


def _schedule():
    """Apps: (d, s0, nb) -> contributes to steps t in [s0+d, s0+d+nb-1]
    using acts_{s0..s0+nb-1}. For d>=2, nb <= d-1 so every app has
    >=1 step of slack and can run NON-fresh (fully off the critical
    path); only d=1 apps ever gate a tanh."""
    apps = []
    for d in range(1, MAXD + 1):
        nsteps = STEPS - d
        den = d if d == 1 else d - 1
        nwin = -(-nsteps // den)
        base, extra = divmod(nsteps, nwin)
        s0 = 1
        for i in range(nwin):
            nb = base + (1 if i < extra else 0)
            apps.append((d, s0, nb))
            s0 += nb
    return apps


def _make_plan(delay_values, connection_indices, rank):
    """Band windows per (d, tc) + greedy app schedule. Core-independent."""
    dl = np.asarray(delay_values)
    ci = np.asarray(connection_indices)
    gs = rank[ci[0].astype(np.int64)] // 128
    gt = rank[ci[1].astype(np.int64)] // 128

    win = {}
    for d in range(1, MAXD + 1):
        m = dl == d
        gsd, gtd = gs[m], gt[m]
        tcd = gtd % 4
        for tc in range(TCH):
            mm = tcd == tc
            if not mm.any():
                win[(d, tc)] = None
                continue
            delta = ((gsd[mm] - gtd[mm] + 16) % 32) - 16
            dmin, dmax = int(delta.min()), int(delta.max())
            if dmax - dmin + 1 >= SCH:
                dmin, dmax = -16, 15
            win[(d, tc)] = (dmin, dmax - dmin + 1)

    wcols = {d: sum(win[(d, tc)][1] for tc in range(TCH)
                    if win[(d, tc)] is not None) * 128
             for d in range(1, MAXD + 1)}
    # lazy weight-load schedule: d>=4 in <=6144-col chunks, 3 queue
    # slots (sync/scalar/gpsimd) per step, in d order. avail[d] = step
    # after the last chunk lands.
    # lazy chunks pinned into each step's mesh window by queue program
    # order; per-queue caps sized so the tail never delays the urgent
    # landing (queues: 0=sync 4096, 1=scalar 4096, 2=gpsimd 6144 cols)
    # lazy chunks on sync+scalar only (a gpsimd chunk sits between the
    # c2 copy and the next cin in queue order and delays the trigger)
    loads = {t: [] for t in range(1, STEPS)}
    avail = {d: 1 for d in range(1, MAXD + 1)}
    free = {(t, qi): (0 if t < 3 else (4864 if t < 4 else 8192))
            for t in range(1, STEPS) for qi in (0, 1)}
    cur_t, cur_q = 1, 0
    for d in LAZY_D:
        c = 0
        while c < wcols[d]:
            while free[(cur_t, cur_q)] < 2048:
                cur_q += 1
                if cur_q == 2:
                    cur_q = 0
                    cur_t += 1
                    assert cur_t < STEPS, "load schedule overflow"
            n = min(free[(cur_t, cur_q)], wcols[d] - c)
            loads[cur_t].append((d, c, c + n, cur_q))
            free[(cur_t, cur_q)] -= n
            c += n
            avail[d] = cur_t + 1
        assert avail[d] <= d, (d, avail[d])

    apps = _schedule()
    appcost = {}
    for (d, s0, nb) in apps:
        appcost[(d, s0, nb)] = sum(win[(d, tc)][1] for tc in range(TCH)
                                   if win[(d, tc)] is not None)
    load = {t: 0 for t in range(1, STEPS)}
    assign = {t: [] for t in range(1, STEPS)}
    apps_sorted = sorted(apps, key=lambda a: ((a[1] + a[0] - 1) -
                                              (a[1] + a[2] - 1),
                                              -appcost[tuple(a)]))
    for (d, s0, nb) in apps_sorted:
        # d>=2: run only once its whole acts window is landed (non-fresh)
        ready = s0 + nb - 1 if d == 1 else s0 + nb
        ready, deadline = max(ready, avail[d]), s0 + d - 1
        assert ready <= deadline, (d, s0, nb, ready, deadline)
        t = min(range(ready, deadline + 1), key=lambda x: load[x])
        load[t] += appcost[(d, s0, nb)]
        assign[t].append((d, s0, nb))
    for t in assign:
        # flexible apps first (they run during the collective); fresh last
        assign[t].sort(key=lambda a, tt=t: (a[1] + a[2] - 1 == tt,
                                            a[1] + a[0] - 1, a[0]))
    # used slot range for the landing (excluding own slots 32..36)
    sts = [tc + SCH + win[(d, tc)][0] for d in range(1, MAXD + 1)
           for tc in range(TCH) if win[(d, tc)] is not None]
    ens = [tc + SCH + win[(d, tc)][0] + win[(d, tc)][1]
           for d in range(1, MAXD + 1)
           for tc in range(TCH) if win[(d, tc)] is not None]
    lo, hi = min(sts), max(ens)
    lo = (lo // TCH) * TCH
    hi = -(-hi // TCH) * TCH
    return win, assign, lo, hi, loads


def _build_program(plan):
    from concourse import bacc, mybir, tile
    from concourse.bass import ds

    win, assign, LO, HI, loads = plan
    dt = mybir.dt
    nc = bacc.Bacc(None, target_bir_lowering=False, debug=False,
                   num_swdge_queues=4)

    wcols = {d: sum(win[(d, tc)][1] for tc in range(TCH)
                    if win[(d, tc)] is not None) * 128
             for d in range(1, MAXD + 1)}
    colbase = {}
    for d in range(1, MAXD + 1):
        c = 0
        for tc in range(TCH):
            colbase[(d, tc)] = c
            if win[(d, tc)] is not None:
                c += win[(d, tc)][1] * 128

    wd_in = {d: nc.declare_dram_parameter(f"wd{d}", [128, wcols[d]],
                                          dt.float8e4, isOutput=False)
             for d in range(1, MAXD + 1)}
    c0r_in = nc.declare_dram_parameter("c0rep", [128, TCH * STEPS * BATCH],
                                       dt.float32, isOutput=False)
    actb1_in = nc.declare_dram_parameter("actb1", [128, TCH * BATCH],
                                         dt.bfloat16, isOutput=False)
    out_d = nc.declare_dram_parameter("out", [128, TCH * BATCH], dt.float32,
                                      isOutput=True)

    # even/odd double-buffered collective bounce ((p,tc)-major rows)
    cin = [nc.dram_tensor(f"cc_in{i}", [TGT_PER_CORE, BATCH], dt.bfloat16)
           for i in range(2)]
    cmid = [nc.dram_tensor(f"cc_mid{i}", [N_NEURONS, BATCH], dt.bfloat16,
                           addr_space="Shared") for i in range(2)]
    cc2 = [nc.dram_tensor(f"cc2_{i}", [3 * N_NEURONS, BATCH], dt.bfloat16)
           for i in range(2)]

    HC = 2 * SCH

    with tile.TileContext(nc) as tc_ctx:
        with (
            tc_ctx.tile_pool(name="wres", bufs=1) as wres_pool,
            tc_ctx.tile_pool(name="aux", bufs=1) as aux_pool,
            tc_ctx.tile_pool(name="psum", bufs=4, space="PSUM") as psum_pool,
        ):
            t_wres = {d: wres_pool.tile([128, wcols[d]], dt.float8e4,
                                        name=f"wres{d}", tag=f"wres{d}")
                      for d in range(1, MAXD + 1)}
            t_acc = aux_pool.tile([128, TCH * STEPS * BATCH], dt.float32)
            t_hist = aux_pool.tile([128, MAXD * HC * BATCH], dt.bfloat16)
            t_actb = aux_pool.tile([128, TCH * BATCH], dt.bfloat16)
            t_act = aux_pool.tile([128, TCH * BATCH], dt.float32)

            # c0 gates tanh_1 -- first on the sync queue
            nc.sync.dma_start(t_acc[:], c0r_in[:])
            # hist must be finite everywhere: edge cores' OOB-skipped
            # urgent landings read (zero-weighted) stale slots
            nc.vector.memset(t_hist[:], 0.0)

            off_eng = {}
            for eng in (nc.sync, nc.scalar, nc.gpsimd):
                off_eng[eng] = eng.partition_id() * TGT_PER_CORE

            hist4 = t_hist[:].rearrange("p (s c r) -> p s c r",
                                        s=MAXD, c=HC)
            acc4 = t_acc[:].rearrange("p (tc t r) -> p tc t r",
                                      tc=TCH, t=STEPS)

            def run_app(d, s0, nb, part):
                """part: 'all' | 'own' | 'halo' (slot subsets)."""
                t_w = t_wres[d]
                tcs = []
                for tc in range(TCH):
                    if win[(d, tc)] is None:
                        continue
                    dmin, W = win[(d, tc)]
                    st = tc + SCH + dmin
                    slots = list(range(W))
                    if part == "own":
                        slots = [i for i in slots if OWN0 <= st + i < OWN1]
                    elif part == "halo":
                        slots = [i for i in slots
                                 if not (OWN0 <= st + i < OWN1)]
                    if slots:
                        tcs.append((tc, st, slots))
                if not tcs:
                    return
                t_scr = psum_pool.tile([128, TCH * MAXB * BATCH], dt.float32,
                                       name="scr", tag="scr")
                scr4 = t_scr[:].rearrange("p (tc b r) -> p tc b r",
                                          tc=TCH, r=BATCH)
                for (tc, st, slots) in tcs:
                    cb = colbase[(d, tc)]
                    for ii, i in enumerate(slots):
                        lhsT = t_w[:, cb + i * 128: cb + (i + 1) * 128]
                        rhs = hist4[:, s0 - 1:s0 - 1 + nb, st + i, :]
                        nc.tensor.matmul(scr4[:, tc, :nb, :], lhsT, rhs,
                                         start=(ii == 0),
                                         stop=(ii == len(slots) - 1))
                t0 = s0 + d
                if len(tcs) == TCH:
                    acc_win = acc4[:, :, t0 - 1:t0 - 1 + nb, :]
                    nc.vector.scalar_tensor_tensor(
                        acc_win, scr4[:, :, :nb, :], 1.0 / FP8_SCALE,
                        acc_win, mybir.AluOpType.mult, mybir.AluOpType.add)
                else:
                    for (tc, _, _) in tcs:
                        acc_win = acc4[:, tc, t0 - 1:t0 - 1 + nb, :]
                        nc.vector.scalar_tensor_tensor(
                            acc_win, scr4[:, tc, :nb, :], 1.0 / FP8_SCALE,
                            acc_win, mybir.AluOpType.mult,
                            mybir.AluOpType.add)

            def emit_collect(t):
                """tanh_t -> cin -> trigger -> own-acts copy.
                cin DMA + trigger share the GpSimd queue: same-queue
                program order replaces a slow DMA-completion wait."""
                par = t % 2
                acc_t = acc4[:, :, t - 1, :]
                nc.scalar.activation(
                    t_actb[:].rearrange("p (tc r) -> p tc r", tc=TCH),
                    acc_t, mybir.ActivationFunctionType.Tanh)
                nc.gpsimd.dma_start(
                    cin[par][:].rearrange("(p f) r -> p (f r)", p=128),
                    t_actb[:])
                nc.gpsimd.collective_compute(
                    "AllGather", mybir.AluOpType.bypass,
                    replica_groups=[list(range(N_CORES))],
                    ins=[cin[par][:]], outs=[cmid[par][:]])
                # own acts straight into rotated history (slots 32..36)
                nc.gpsimd.tensor_copy(
                    hist4[:, t - 1, OWN0:OWN1, :],
                    t_actb[:].rearrange("p (tc r) -> p tc r", tc=TCH))

            # urgent-landing band: the d=1 halo slots. Edge cores' slots
            # fall outside [0, N) in global coords -> their weight tiles
            # are all-zero, so an OOB-skipped DMA (stale data) is correct.
            U0 = min(tc + SCH + win[(1, tc)][0] for tc in range(TCH)
                     if win[(1, tc)] is not None)
            U1 = max(tc + SCH + win[(1, tc)][0] + win[(1, tc)][1]
                     for tc in range(TCH) if win[(1, tc)] is not None)
            bel_n, abv_n = OWN0 - U0, U1 - OWN1
            assert 0 < bel_n <= TCH and 0 < abv_n <= TCH, (bel_n, abv_n)

            urg_off = {}
            for (eng, dr) in ((nc.sync, N_CORES - 1), (nc.scalar, 1)):
                urg_off[eng] = ((eng.partition_id() + dr) % N_CORES) * \
                    TGT_PER_CORE

            def emit_urgent(t):
                """d=1 halo slots straight from cmid (no bounce). The
                neighbour rank wraps around the ring at the edges; the
                wrapped block hits all-zero weight tiles, so any finite
                data there is correct."""
                par = t % 2
                for (eng, fsl, slo, n) in (
                        (nc.sync, TCH - bel_n, U0, bel_n),
                        (nc.scalar, 0, OWN1, abv_n)):
                    src = cmid[par][ds(urg_off[eng], TGT_PER_CORE),
                                    :].rearrange("(p f) r -> p f r", f=TCH)
                    eng.dma_start(hist4[:, t - 1, slo:slo + n, :],
                                  src[:, fsl:fsl + n, :])

            def emit_landing(t):
                par = t % 2
                c2 = cc2[par]
                nc.sync.dma_start(c2[0:N_NEURONS, :], cmid[par][:])
                nc.scalar.dma_start(c2[N_NEURONS:2 * N_NEURONS, :],
                                    cmid[par][:])
                nc.gpsimd.dma_start(c2[2 * N_NEURONS:3 * N_NEURONS, :],
                                    cmid[par][:])
                # trimmed rotated landing, skipping own slots
                dst = hist4[:, t - 1, :, :].rearrange(
                    "p (j f) r -> p j (f r)", f=TCH)
                for (c0_, c1_, eng) in ((LO, OWN0, nc.sync),
                                        (OWN1, HI, nc.scalar)):
                    n = c1_ - c0_
                    if n <= 0:
                        continue
                    src = c2[ds(off_eng[eng] + c0_ * 128, n * 128),
                             :].rearrange("(j p f) r -> p j (f r)",
                                          p=128, f=TCH)
                    eng.dma_start(dst[:, c0_ // TCH: c1_ // TCH, :], src)

            # ---- step 1 prologue: acts_1 = tanh(c0) is HOST-computed
            # (actb1), so cin_1 + trigger_1 have no on-device deps and
            # fire ~1us in -- the fabric warm-up starts immediately ----
            sc1 = nc.named_scope("step01")
            sc1.__enter__()
            nc.gpsimd.dma_start(
                cin[1][:].rearrange("(p f) r -> p (f r)", p=128),
                actb1_in[:])
            nc.gpsimd.collective_compute(
                "AllGather", mybir.AluOpType.bypass,
                replica_groups=[list(range(N_CORES))],
                ins=[cin[1][:]], outs=[cmid[1][:]])
            # own acts_1 into SBUF actb + rotated hist
            nc.scalar.dma_start(t_actb[:], actb1_in[:])
            nc.gpsimd.tensor_copy(
                hist4[:, 0, OWN0:OWN1, :],
                t_actb[:].rearrange("p (tc r) -> p tc r", tc=TCH))
            sc1.__exit__(None, None, None)

            # bulk weight preloads: every core idles ~80us in the first
            # collective's fabric warm-up; these drain there (they are
            # ready at t=0 while all AG-gated ops block)
            pre_d = (1, 2, 3, 4, 5, 7, 8, 9, 12, 13, 14)
            pengs = (nc.sync, nc.scalar, nc.gpsimd)
            pieces = []
            for d in pre_d:
                for c in range(0, wcols[d], 4096):
                    pieces.append((d, c, min(c + 4096, wcols[d])))
            for i, (d, ca, cb) in enumerate(pieces):
                pengs[i % 3].dma_start(t_wres[d][:, ca:cb],
                                       wd_in[d][:, ca:cb])
            lqueues = (nc.sync, nc.scalar, nc.gpsimd)
            for t in range(1, STEPS + 1):
                sc_ctx = nc.named_scope(f"step{t:02d}")
                sc_ctx.__enter__()
                if t == STEPS:
                    nc.scalar.activation(
                        t_act[:].rearrange("p (tc r) -> p tc r", tc=TCH),
                        acc4[:, :, t - 1, :],
                        mybir.ActivationFunctionType.Tanh)
                    nc.sync.dma_start(out_d[:], t_act[:])
                    sc_ctx.__exit__(None, None, None)
                    break
                if t > 1:
                    emit_collect(t)
                # full landing of the PREVIOUS step's gather + weight
                # chunks: pinned (by queue program order) into THIS
                # step's mesh window, after cin+trigger, keeping their
                # DMA traffic off the post-AG critical chain
                if t > 1:
                    with nc.named_scope(f"land{t - 1:02d}"):
                        emit_landing(t - 1)
                for (d, ca, cb, qi) in loads[t]:
                    lqueues[qi].dma_start(t_wres[d][:, ca:cb],
                                          wd_in[d][:, ca:cb])
                emit_urgent(t)
                sc_ctx.__exit__(None, None, None)
                fresh = [a for a in assign[t] if a[0] == 1 and
                         a[1] + a[2] - 1 == t]
                # deadline-assigned flexible apps gate tanh_{t+1}; they
                # only need already-landed history, so run them on the PE
                # DURING this step's AllGather, ahead of the fresh app
                gating = [a for a in assign[t] if a not in fresh and
                          a[1] + a[0] - 1 == t]
                rest = [a for a in assign[t] if a not in fresh and
                        a not in gating]
                for (d, s0, nb) in gating:
                    with nc.named_scope(f"app_d{d}_s{s0}"):
                        run_app(d, s0, nb, "all")
                # critical chain: the fresh d=1 app (own reads the gpsimd
                # own-copy; halo reads the urgent landing)
                for (d, s0, nb) in fresh:
                    with nc.named_scope(f"app_d{d}_s{s0}"):
                        run_app(d, s0, nb, "own")
                        run_app(d, s0, nb, "halo")
                for (d, s0, nb) in rest:
                    with nc.named_scope(f"app_d{d}_s{s0}"):
                        run_app(d, s0, nb, "all")

    nc.compile()
    return nc


def _spatial_rank(connection_indices, delay_values):
    """Estimate 1D landmark projection from graph distances; return rank."""
    import scipy.sparse as sp
    from scipy.sparse.csgraph import dijkstra
    ci = np.asarray(connection_indices)
    dl = np.asarray(delay_values)
    src = ci[0].astype(np.int64)
    tgt = ci[1].astype(np.int64)
    w = dl.astype(np.float64) + 0.5
    rr = np.concatenate([src, tgt])
    cc = np.concatenate([tgt, src])
    ww = np.concatenate([w, w])
    order = np.lexsort((cc, rr))
    rr, cc, ww = rr[order], cc[order], ww[order]
    same = (rr[1:] == rr[:-1]) & (cc[1:] == cc[:-1])
    starts = np.flatnonzero(np.concatenate([[True], ~same]))
    wmin = np.minimum.reduceat(ww, starts)
    G = sp.csr_matrix((wmin, (rr[starts], cc[starts])),
                      shape=(N_NEURONS, N_NEURONS))
    D0 = dijkstra(G, indices=0)
    t1 = int(np.argmax(D0))
    D1 = dijkstra(G, indices=t1)
    t2 = int(np.argmax(D1))
    D2 = dijkstra(G, indices=t2)
    proj = (D1 ** 2 - D2 ** 2) / (2.0 * max(D1[t2], 1e-9))
    pi = np.argsort(proj, kind="stable")
    rank = np.empty(N_NEURONS, np.int64)
    rank[pi] = np.arange(N_NEURONS)
    return pi, rank


def _preprocess(input_data, connection_weights, connection_indices,
                delay_values, steps):
    """Host: permutation, banded per-core weights, c0, plan."""
    import ml_dtypes
    assert steps == STEPS
    w = np.asarray(connection_weights, np.float32)
    ci = np.asarray(connection_indices)
    dl = np.asarray(delay_values)
    x = np.asarray(input_data, np.float32)

    pi, rank = _spatial_rank(ci, dl)
    plan = _make_plan(dl, ci, rank)
    win = plan[0]

    src = rank[ci[0].astype(np.int64)]
    tgt = rank[ci[1].astype(np.int64)]

    acts0 = np.zeros((BATCH, N_NEURONS), np.float32)
    acts0[:, :INPUT_SIZE] = x
    acts0 = acts0[:, pi]

    m0 = dl == 0
    c0 = np.zeros((BATCH, N_NEURONS), np.float32)
    for r in range(BATCH):
        np.add.at(c0[r], tgt[m0], w[m0] * acts0[r, src[m0]])

    wds = {}
    for d in range(1, MAXD + 1):
        md = dl == d
        Wd = np.zeros((N_NEURONS, N_NEURONS), np.float32)
        np.add.at(Wd, (src[md], tgt[md]), w[md])
        wds[d] = (Wd * FP8_SCALE).astype(ml_dtypes.float8_e4m3fn)

    in_maps = []
    for k in range(N_CORES):
        im = {}
        for d in range(1, MAXD + 1):
            cols = []
            for tc in range(TCH):
                if win[(d, tc)] is None:
                    continue
                dmin, W = win[(d, tc)]
                gt_glob = 4 * k + tc
                t0c = gt_glob * 128
                for i in range(W):
                    gc = (gt_glob + dmin + i) % SCH
                    cols.append(wds[d][gc * 128:(gc + 1) * 128,
                                       t0c:t0c + 128])
            Wp = np.concatenate(cols, axis=1) if cols else \
                np.zeros((128, 0), ml_dtypes.float8_e4m3fn)
            im[f"wd{d}"] = np.ascontiguousarray(Wp)
        t0 = k * TGT_PER_CORE
        c0r = np.zeros((128, TCH, STEPS, BATCH), np.float32)
        for tci in range(TCH):
            for r in range(BATCH):
                c0r[:, tci, :, r] = c0[r, t0 + tci * 128:
                                       t0 + (tci + 1) * 128][:, None]
        im["c0rep"] = c0r.reshape(128, TCH * STEPS * BATCH)
        a1 = np.tanh(c0)  # acts_1, host-computed
        ab = np.zeros((128, TCH, BATCH), np.float32)
        for tci in range(TCH):
            for r in range(BATCH):
                ab[:, tci, r] = a1[r, t0 + tci * 128: t0 + (tci + 1) * 128]
        im["actb1"] = ab.reshape(128, TCH * BATCH).astype(ml_dtypes.bfloat16)
        in_maps.append(im)
    return in_maps, plan


def kernel(input_data, connection_weights, connection_indices,
           delay_values, steps):
    global _compiled, _compiled_key
    from concourse.bass_utils import run_bass_kernel_spmd

    in_maps, plan = _preprocess(input_data, connection_weights,
                                connection_indices, delay_values, int(steps))
    key = repr(plan[0])
    if _compiled is None or _compiled_key != key:
        _compiled = _build_program(plan)
        _compiled_key = key
    res = run_bass_kernel_spmd(_compiled, in_maps, list(range(N_CORES)))

    pi, _ = _spatial_rank(connection_indices, delay_values)
    out_rank = np.zeros((BATCH, N_NEURONS), np.float32)
    for k in range(N_CORES):
        o = res.results[k]["out"]
        t0 = k * TGT_PER_CORE
        for tci in range(TCH):
            for r in range(BATCH):
                out_rank[r, t0 + tci * 128: t0 + (tci + 1) * 128] = \
                    o[:, tci * BATCH + r]
    out = np.zeros((BATCH, N_NEURONS), np.float32)
    out[:, pi] = out_rank
    return out[:, -INPUT_SIZE:].astype(np.float32)


# revision 46
# speedup vs baseline: 1.0686x; 1.0686x over previous
"""TRN2 Bass kernel for nn_Brain: delayed-synapse recurrent network.

Strategy (banded delay-batched futures, v4):
  total_input[t] = c0 + sum_{d=1}^{15} W_d @ acts_{t-d};  acts_t = tanh(...)
- Edges with delay >= 16 never fire; delay-0 edges give a per-neuron
  constant c0 (host-computed).
- Neurons are SPATIALLY SORTED: delays are int(Euclidean distance) of
  points in a cube, so graph distances (Dijkstra with edge length d+0.5)
  recover a 1D landmark projection; sorting by it makes every W_d BANDED.
  Out-of-band 128x128 weight tiles are skipped.
- Targets sharded contiguously in sorted order (512/core). SPMD identical
  program: per-core band windows become uniform by keeping the activation
  history in CORE-ROTATED chunk coordinates (rotated chunk c holds global
  chunk (4k + c) mod 32 on core k; own chunks always at slots 32..36 of
  the doubled space). The per-step AllGather lands via DynSlice
  (partition_id-offset) DMA windows from a 3x-copied DRAM bounce buffer.
- Latency structure (measured): DMA-completion semaphores post 1.4-4us
  after the wire drains (worse under concurrent DMA); engine-to-engine
  event hops are ~0.1us; the 8-core AllGather mesh is ~8.3us
  trigger-to-done and inflates under concurrent DMA traffic; the FIRST
  collective pays a ~75-95us fabric warm-up that every core idles
  through. Hence:
  * acts_1 = tanh(c0) is host-computed, so step-1's cin is a DRAM-DRAM
    copy with no on-device deps: trigger_1 fires ~10us in and the
    warm-up clock starts immediately.
  * ~15MB of the 22.2MB SBUF-resident banded weights preload during
    the warm-up window; the rest trickles in capped sync/scalar chunks
    (never gpsimd: a chunk there would sit between the c2 copy and the
    next cin in queue order and delay the trigger).
  * Per step, only the d=1 fresh app gates tanh_{t+1}. Its halo slots
    land DIRECTLY from the AllGather output (cmid) via a ring-wrapped
    DynSlice window (edge cores read a wrong-rank block that hits
    all-zero weight tiles). The full 3x-bounce landing is deferred
    until after the NEXT step's cin+trigger, off the critical chain.
  * d>=2 apps use nb<=d-1 windows so they are never fresh; the ones
    whose deadline gates tanh_{t+1} are emitted first and run on the
    PE during the AllGather.
  * cin DMA + trigger + own-acts copy share the GpSimd queue.
- Bucket d applies in windows of nb steps (both batch rows ride as
  matmul columns).
"""
import numpy as np

N_NEURONS = 4096
INPUT_SIZE = 1024
BATCH = 2
STEPS = 16
N_CORES = 8
TGT_PER_CORE = N_NEURONS // N_CORES        # 512
TCH = TGT_PER_CORE // 128                  # 4 target chunks per core
SCH = N_NEURONS // 128                     # 32 global source chunks
MAXD = STEPS - 1                           # delays 1..15 useful
FP8_SCALE = 64.0
MAXB = 8
OWN0, OWN1 = SCH, SCH + TCH                # own chunks: rotated slots 32..36
# weights loaded lazily during steps; the rest preload during the ~80us
# first-collective fabric warm-up that every core sits through
LAZY_D = (6, 10, 11, 15)

_compiled = None
_compiled_key = None


def _schedule():
    """Apps: (d, s0, nb) -> contributes to steps t in [s0+d, s0+d+nb-1]
    using acts_{s0..s0+nb-1}. For d>=2, nb <= d-1 so every app has
    >=1 step of slack and can run NON-fresh (fully off the critical
    path); only d=1 apps ever gate a tanh."""
    apps = []
    for d in range(1, MAXD + 1):
        nsteps = STEPS - d
        den = d if d == 1 else d - 1
        nwin = -(-nsteps // den)
        base, extra = divmod(nsteps, nwin)
        s0 = 1
        for i in range(nwin):
            nb = base + (1 if i < extra else 0)
            apps.append((d, s0, nb))
            s0 += nb
    return apps


def _make_plan(delay_values, connection_indices, rank):
    """Band windows per (d, tc) + greedy app schedule. Core-independent."""
    dl = np.asarray(delay_values)
    ci = np.asarray(connection_indices)
    gs = rank[ci[0].astype(np.int64)] // 128
    gt = rank[ci[1].astype(np.int64)] // 128

    win = {}
    for d in range(1, MAXD + 1):
        m = dl == d
        gsd, gtd = gs[m], gt[m]
        tcd = gtd % 4
        for tc in range(TCH):
            mm = tcd == tc
            if not mm.any():
                win[(d, tc)] = None
                continue
            delta = ((gsd[mm] - gtd[mm] + 16) % 32) - 16
            dmin, dmax = int(delta.min()), int(delta.max())
            if dmax - dmin + 1 >= SCH:
                dmin, dmax = -16, 15
            win[(d, tc)] = (dmin, dmax - dmin + 1)

    wcols = {d: sum(win[(d, tc)][1] for tc in range(TCH)
                    if win[(d, tc)] is not None) * 128
             for d in range(1, MAXD + 1)}
    # lazy weight-load schedule: d>=4 in <=6144-col chunks, 3 queue
    # slots (sync/scalar/gpsimd) per step, in d order. avail[d] = step
    # after the last chunk lands.
    # lazy chunks pinned into each step's mesh window by queue program
    # order; per-queue caps sized so the tail never delays the urgent
    # landing (queues: 0=sync 4096, 1=scalar 4096, 2=gpsimd 6144 cols)
    # lazy chunks on sync+scalar only (a gpsimd chunk sits between the
    # c2 copy and the next cin in queue order and delays the trigger)
    loads = {t: [] for t in range(1, STEPS)}
    avail = {d: 1 for d in range(1, MAXD + 1)}
    free = {(t, qi): (0 if t < 2 else (4864 if t < 4 else 8192))
            for t in range(1, STEPS) for qi in (0, 1)}
    cur_t, cur_q = 1, 0
    for d in LAZY_D:
        c = 0
        while c < wcols[d]:
            while free[(cur_t, cur_q)] < 2048:
                cur_q += 1
                if cur_q == 2:
                    cur_q = 0
                    cur_t += 1
                    assert cur_t < STEPS, "load schedule overflow"
            n = min(free[(cur_t, cur_q)], wcols[d] - c)
            loads[cur_t].append((d, c, c + n, cur_q))
            free[(cur_t, cur_q)] -= n
            c += n
            avail[d] = cur_t + 1
        assert avail[d] <= d, (d, avail[d])

    apps = _schedule()
    appcost = {}
    for (d, s0, nb) in apps:
        appcost[(d, s0, nb)] = sum(win[(d, tc)][1] for tc in range(TCH)
                                   if win[(d, tc)] is not None)
    load = {t: 0 for t in range(1, STEPS)}
    assign = {t: [] for t in range(1, STEPS)}
    apps_sorted = sorted(apps, key=lambda a: ((a[1] + a[0] - 1) -
                                              (a[1] + a[2] - 1),
                                              -appcost[tuple(a)]))
    for (d, s0, nb) in apps_sorted:
        # d>=2: run only once its whole acts window is landed (non-fresh)
        ready = s0 + nb - 1 if d == 1 else s0 + nb
        ready, deadline = max(ready, avail[d]), s0 + d - 1
        assert ready <= deadline, (d, s0, nb, ready, deadline)
        t = min(range(ready, deadline + 1), key=lambda x: load[x])
        load[t] += appcost[(d, s0, nb)]
        assign[t].append((d, s0, nb))
    for t in assign:
        # flexible apps first (they run during the collective); fresh last
        assign[t].sort(key=lambda a, tt=t: (a[1] + a[2] - 1 == tt,
                                            a[1] + a[0] - 1, a[0]))
    # used slot range for the landing (excluding own slots 32..36)
    sts = [tc + SCH + win[(d, tc)][0] for d in range(1, MAXD + 1)
           for tc in range(TCH) if win[(d, tc)] is not None]
    ens = [tc + SCH + win[(d, tc)][0] + win[(d, tc)][1]
           for d in range(1, MAXD + 1)
           for tc in range(TCH) if win[(d, tc)] is not None]
    lo, hi = min(sts), max(ens)
    lo = (lo // TCH) * TCH
    hi = -(-hi // TCH) * TCH
    return win, assign, lo, hi, loads


def _build_program(plan):
    from concourse import bacc, mybir, tile
    from concourse.bass import ds

    win, assign, LO, HI, loads = plan
    dt = mybir.dt
    nc = bacc.Bacc(None, target_bir_lowering=False, debug=False,
                   num_swdge_queues=4)

    wcols = {d: sum(win[(d, tc)][1] for tc in range(TCH)
                    if win[(d, tc)] is not None) * 128
             for d in range(1, MAXD + 1)}
    colbase = {}
    for d in range(1, MAXD + 1):
        c = 0
        for tc in range(TCH):
            colbase[(d, tc)] = c
            if win[(d, tc)] is not None:
                c += win[(d, tc)][1] * 128

    wd_in = {d: nc.declare_dram_parameter(f"wd{d}", [128, wcols[d]],
                                          dt.float8e4, isOutput=False)
             for d in range(1, MAXD + 1)}
    c0r_in = nc.declare_dram_parameter("c0rep", [128, TCH * STEPS * BATCH],
                                       dt.float32, isOutput=False)
    actb1_in = nc.declare_dram_parameter("actb1", [128, TCH * BATCH],
                                         dt.bfloat16, isOutput=False)
    out_d = nc.declare_dram_parameter("out", [128, TCH * BATCH], dt.float32,
                                      isOutput=True)

    # even/odd double-buffered collective bounce ((p,tc)-major rows)
    cin = [nc.dram_tensor(f"cc_in{i}", [TGT_PER_CORE, BATCH], dt.bfloat16)
           for i in range(2)]
    cmid = [nc.dram_tensor(f"cc_mid{i}", [N_NEURONS, BATCH], dt.bfloat16,
                           addr_space="Shared") for i in range(2)]
    cc2 = [nc.dram_tensor(f"cc2_{i}", [3 * N_NEURONS, BATCH], dt.bfloat16)
           for i in range(2)]

    HC = 2 * SCH

    with tile.TileContext(nc) as tc_ctx:
        with (
            tc_ctx.tile_pool(name="wres", bufs=1) as wres_pool,
            tc_ctx.tile_pool(name="aux", bufs=1) as aux_pool,
            tc_ctx.tile_pool(name="psum", bufs=4, space="PSUM") as psum_pool,
        ):
            t_wres = {d: wres_pool.tile([128, wcols[d]], dt.float8e4,
                                        name=f"wres{d}", tag=f"wres{d}")
                      for d in range(1, MAXD + 1)}
            t_acc = aux_pool.tile([128, TCH * STEPS * BATCH], dt.float32)
            t_hist = aux_pool.tile([128, MAXD * HC * BATCH], dt.bfloat16)
            t_actb = aux_pool.tile([128, TCH * BATCH], dt.bfloat16)
            t_act = aux_pool.tile([128, TCH * BATCH], dt.float32)

            # c0 gates tanh_1 -- first on the sync queue
            nc.sync.dma_start(t_acc[:], c0r_in[:])
            # hist must be finite everywhere: edge cores' OOB-skipped
            # urgent landings read (zero-weighted) stale slots
            nc.vector.memset(t_hist[:], 0.0)

            off_eng = {}
            for eng in (nc.sync, nc.scalar, nc.gpsimd):
                off_eng[eng] = eng.partition_id() * TGT_PER_CORE

            hist4 = t_hist[:].rearrange("p (s c r) -> p s c r",
                                        s=MAXD, c=HC)
            acc4 = t_acc[:].rearrange("p (tc t r) -> p tc t r",
                                      tc=TCH, t=STEPS)

            def run_app(d, s0, nb, part):
                """part: 'all' | 'own' | 'halo' (slot subsets)."""
                t_w = t_wres[d]
                tcs = []
                for tc in range(TCH):
                    if win[(d, tc)] is None:
                        continue
                    dmin, W = win[(d, tc)]
                    st = tc + SCH + dmin
                    slots = list(range(W))
                    if part == "own":
                        slots = [i for i in slots if OWN0 <= st + i < OWN1]
                    elif part == "halo":
                        slots = [i for i in slots
                                 if not (OWN0 <= st + i < OWN1)]
                    if slots:
                        tcs.append((tc, st, slots))
                if not tcs:
                    return
                t_scr = psum_pool.tile([128, TCH * MAXB * BATCH], dt.float32,
                                       name="scr", tag="scr")
                scr4 = t_scr[:].rearrange("p (tc b r) -> p tc b r",
                                          tc=TCH, r=BATCH)
                for (tc, st, slots) in tcs:
                    cb = colbase[(d, tc)]
                    for ii, i in enumerate(slots):
                        lhsT = t_w[:, cb + i * 128: cb + (i + 1) * 128]
                        rhs = hist4[:, s0 - 1:s0 - 1 + nb, st + i, :]
                        nc.tensor.matmul(scr4[:, tc, :nb, :], lhsT, rhs,
                                         start=(ii == 0),
                                         stop=(ii == len(slots) - 1))
                t0 = s0 + d
                if len(tcs) == TCH:
                    acc_win = acc4[:, :, t0 - 1:t0 - 1 + nb, :]
                    nc.vector.scalar_tensor_tensor(
                        acc_win, scr4[:, :, :nb, :], 1.0 / FP8_SCALE,
                        acc_win, mybir.AluOpType.mult, mybir.AluOpType.add)
                else:
                    for (tc, _, _) in tcs:
                        acc_win = acc4[:, tc, t0 - 1:t0 - 1 + nb, :]
                        nc.vector.scalar_tensor_tensor(
                            acc_win, scr4[:, tc, :nb, :], 1.0 / FP8_SCALE,
                            acc_win, mybir.AluOpType.mult,
                            mybir.AluOpType.add)

            def emit_collect(t):
                """tanh_t -> cin -> trigger -> own-acts copy.
                cin DMA + trigger share the GpSimd queue: same-queue
                program order replaces a slow DMA-completion wait."""
                par = t % 2
                acc_t = acc4[:, :, t - 1, :]
                nc.scalar.activation(
                    t_actb[:].rearrange("p (tc r) -> p tc r", tc=TCH),
                    acc_t, mybir.ActivationFunctionType.Tanh)
                nc.gpsimd.dma_start(
                    cin[par][:].rearrange("(p f) r -> p (f r)", p=128),
                    t_actb[:])
                nc.gpsimd.collective_compute(
                    "AllGather", mybir.AluOpType.bypass,
                    replica_groups=[list(range(N_CORES))],
                    ins=[cin[par][:]], outs=[cmid[par][:]])
                # own acts straight into rotated history (slots 32..36)
                nc.gpsimd.tensor_copy(
                    hist4[:, t - 1, OWN0:OWN1, :],
                    t_actb[:].rearrange("p (tc r) -> p tc r", tc=TCH))

            # urgent-landing band: the d=1 halo slots. Edge cores' slots
            # fall outside [0, N) in global coords -> their weight tiles
            # are all-zero, so an OOB-skipped DMA (stale data) is correct.
            U0 = min(tc + SCH + win[(1, tc)][0] for tc in range(TCH)
                     if win[(1, tc)] is not None)
            U1 = max(tc + SCH + win[(1, tc)][0] + win[(1, tc)][1]
                     for tc in range(TCH) if win[(1, tc)] is not None)
            bel_n, abv_n = OWN0 - U0, U1 - OWN1
            assert 0 < bel_n <= TCH and 0 < abv_n <= TCH, (bel_n, abv_n)

            urg_off = {}
            for (eng, dr) in ((nc.sync, N_CORES - 1), (nc.scalar, 1)):
                urg_off[eng] = ((eng.partition_id() + dr) % N_CORES) * \
                    TGT_PER_CORE

            def emit_urgent(t):
                """d=1 halo slots straight from cmid (no bounce). The
                neighbour rank wraps around the ring at the edges; the
                wrapped block hits all-zero weight tiles, so any finite
                data there is correct."""
                par = t % 2
                for (eng, fsl, slo, n) in (
                        (nc.sync, TCH - bel_n, U0, bel_n),
                        (nc.scalar, 0, OWN1, abv_n)):
                    src = cmid[par][ds(urg_off[eng], TGT_PER_CORE),
                                    :].rearrange("(p f) r -> p f r", f=TCH)
                    eng.dma_start(hist4[:, t - 1, slo:slo + n, :],
                                  src[:, fsl:fsl + n, :])

            def emit_landing(t):
                par = t % 2
                c2 = cc2[par]
                nc.sync.dma_start(c2[0:N_NEURONS, :], cmid[par][:])
                nc.scalar.dma_start(c2[N_NEURONS:2 * N_NEURONS, :],
                                    cmid[par][:])
                nc.gpsimd.dma_start(c2[2 * N_NEURONS:3 * N_NEURONS, :],
                                    cmid[par][:])
                # trimmed rotated landing, skipping own slots
                dst = hist4[:, t - 1, :, :].rearrange(
                    "p (j f) r -> p j (f r)", f=TCH)
                for (c0_, c1_, eng) in ((LO, OWN0, nc.sync),
                                        (OWN1, HI, nc.scalar)):
                    n = c1_ - c0_
                    if n <= 0:
                        continue
                    src = c2[ds(off_eng[eng] + c0_ * 128, n * 128),
                             :].rearrange("(j p f) r -> p j (f r)",
                                          p=128, f=TCH)
                    eng.dma_start(dst[:, c0_ // TCH: c1_ // TCH, :], src)

            # ---- step 1 prologue: acts_1 = tanh(c0) is HOST-computed
            # (actb1), so cin_1 + trigger_1 have no on-device deps and
            # fire ~1us in -- the fabric warm-up starts immediately ----
            sc1 = nc.named_scope("step01")
            sc1.__enter__()
            nc.gpsimd.dma_start(
                cin[1][:].rearrange("(p f) r -> p (f r)", p=128),
                actb1_in[:])
            nc.gpsimd.collective_compute(
                "AllGather", mybir.AluOpType.bypass,
                replica_groups=[list(range(N_CORES))],
                ins=[cin[1][:]], outs=[cmid[1][:]])
            # own acts_1 into SBUF actb + rotated hist
            nc.scalar.dma_start(t_actb[:], actb1_in[:])
            nc.gpsimd.tensor_copy(
                hist4[:, 0, OWN0:OWN1, :],
                t_actb[:].rearrange("p (tc r) -> p tc r", tc=TCH))
            sc1.__exit__(None, None, None)

            # bulk weight preloads: every core idles ~80us in the first
            # collective's fabric warm-up; these drain there (they are
            # ready at t=0 while all AG-gated ops block)
            preload = {nc.sync: (1, 2, 3, 9, 12), nc.scalar: (4, 7, 13),
                       nc.gpsimd: (5, 8, 14)}
            for eng, ds_list in preload.items():
                for d in ds_list:
                    eng.dma_start(t_wres[d][:], wd_in[d][:])
            lqueues = (nc.sync, nc.scalar, nc.gpsimd)
            for t in range(1, STEPS + 1):
                sc_ctx = nc.named_scope(f"step{t:02d}")
                sc_ctx.__enter__()
                if t == STEPS:
                    nc.scalar.activation(
                        t_act[:].rearrange("p (tc r) -> p tc r", tc=TCH),
                        acc4[:, :, t - 1, :],
                        mybir.ActivationFunctionType.Tanh)
                    nc.sync.dma_start(out_d[:], t_act[:])
                    sc_ctx.__exit__(None, None, None)
                    break
                if t > 1:
                    emit_collect(t)
                # full landing of the PREVIOUS step's gather + weight
                # chunks: pinned (by queue program order) into THIS
                # step's mesh window, after cin+trigger, keeping their
                # DMA traffic off the post-AG critical chain
                if t > 1:
                    with nc.named_scope(f"land{t - 1:02d}"):
                        emit_landing(t - 1)
                for (d, ca, cb, qi) in loads[t]:
                    lqueues[qi].dma_start(t_wres[d][:, ca:cb],
                                          wd_in[d][:, ca:cb])
                emit_urgent(t)
                sc_ctx.__exit__(None, None, None)
                fresh = [a for a in assign[t] if a[0] == 1 and
                         a[1] + a[2] - 1 == t]
                # deadline-assigned flexible apps gate tanh_{t+1}; they
                # only need already-landed history, so run them on the PE
                # DURING this step's AllGather, ahead of the fresh app
                gating = [a for a in assign[t] if a not in fresh and
                          a[1] + a[0] - 1 == t]
                rest = [a for a in assign[t] if a not in fresh and
                        a not in gating]
                for (d, s0, nb) in gating:
                    with nc.named_scope(f"app_d{d}_s{s0}"):
                        run_app(d, s0, nb, "all")
                # critical chain: the fresh d=1 app (own reads the gpsimd
                # own-copy; halo reads the urgent landing)
                for (d, s0, nb) in fresh:
                    with nc.named_scope(f"app_d{d}_s{s0}"):
                        run_app(d, s0, nb, "own")
                        run_app(d, s0, nb, "halo")
                for (d, s0, nb) in rest:
                    with nc.named_scope(f"app_d{d}_s{s0}"):
                        run_app(d, s0, nb, "all")

    nc.compile()
    return nc


def _spatial_rank(connection_indices, delay_values):
    """Estimate 1D landmark projection from graph distances; return rank."""
    import scipy.sparse as sp
    from scipy.sparse.csgraph import dijkstra
    ci = np.asarray(connection_indices)
    dl = np.asarray(delay_values)
    src = ci[0].astype(np.int64)
    tgt = ci[1].astype(np.int64)
    w = dl.astype(np.float64) + 0.5
    rr = np.concatenate([src, tgt])
    cc = np.concatenate([tgt, src])
    ww = np.concatenate([w, w])
    order = np.lexsort((cc, rr))
    rr, cc, ww = rr[order], cc[order], ww[order]
    same = (rr[1:] == rr[:-1]) & (cc[1:] == cc[:-1])
    starts = np.flatnonzero(np.concatenate([[True], ~same]))
    wmin = np.minimum.reduceat(ww, starts)
    G = sp.csr_matrix((wmin, (rr[starts], cc[starts])),
                      shape=(N_NEURONS, N_NEURONS))
    D0 = dijkstra(G, indices=0)
    t1 = int(np.argmax(D0))
    D1 = dijkstra(G, indices=t1)
    t2 = int(np.argmax(D1))
    D2 = dijkstra(G, indices=t2)
    proj = (D1 ** 2 - D2 ** 2) / (2.0 * max(D1[t2], 1e-9))
    pi = np.argsort(proj, kind="stable")
    rank = np.empty(N_NEURONS, np.int64)
    rank[pi] = np.arange(N_NEURONS)
    return pi, rank


def _preprocess(input_data, connection_weights, connection_indices,
                delay_values, steps):
    """Host: permutation, banded per-core weights, c0, plan."""
    import ml_dtypes
    assert steps == STEPS
    w = np.asarray(connection_weights, np.float32)
    ci = np.asarray(connection_indices)
    dl = np.asarray(delay_values)
    x = np.asarray(input_data, np.float32)

    pi, rank = _spatial_rank(ci, dl)
    plan = _make_plan(dl, ci, rank)
    win = plan[0]

    src = rank[ci[0].astype(np.int64)]
    tgt = rank[ci[1].astype(np.int64)]

    acts0 = np.zeros((BATCH, N_NEURONS), np.float32)
    acts0[:, :INPUT_SIZE] = x
    acts0 = acts0[:, pi]

    m0 = dl == 0
    c0 = np.zeros((BATCH, N_NEURONS), np.float32)
    for r in range(BATCH):
        np.add.at(c0[r], tgt[m0], w[m0] * acts0[r, src[m0]])

    wds = {}
    for d in range(1, MAXD + 1):
        md = dl == d
        Wd = np.zeros((N_NEURONS, N_NEURONS), np.float32)
        np.add.at(Wd, (src[md], tgt[md]), w[md])
        wds[d] = (Wd * FP8_SCALE).astype(ml_dtypes.float8_e4m3fn)

    in_maps = []
    for k in range(N_CORES):
        im = {}
        for d in range(1, MAXD + 1):
            cols = []
            for tc in range(TCH):
                if win[(d, tc)] is None:
                    continue
                dmin, W = win[(d, tc)]
                gt_glob = 4 * k + tc
                t0c = gt_glob * 128
                for i in range(W):
                    gc = (gt_glob + dmin + i) % SCH
                    cols.append(wds[d][gc * 128:(gc + 1) * 128,
                                       t0c:t0c + 128])
            Wp = np.concatenate(cols, axis=1) if cols else \
                np.zeros((128, 0), ml_dtypes.float8_e4m3fn)
            im[f"wd{d}"] = np.ascontiguousarray(Wp)
        t0 = k * TGT_PER_CORE
        c0r = np.zeros((128, TCH, STEPS, BATCH), np.float32)
        for tci in range(TCH):
            for r in range(BATCH):
                c0r[:, tci, :, r] = c0[r, t0 + tci * 128:
                                       t0 + (tci + 1) * 128][:, None]
        im["c0rep"] = c0r.reshape(128, TCH * STEPS * BATCH)
        a1 = np.tanh(c0)  # acts_1, host-computed
        ab = np.zeros((128, TCH, BATCH), np.float32)
        for tci in range(TCH):
            for r in range(BATCH):
                ab[:, tci, r] = a1[r, t0 + tci * 128: t0 + (tci + 1) * 128]
        im["actb1"] = ab.reshape(128, TCH * BATCH).astype(ml_dtypes.bfloat16)
        in_maps.append(im)
    return in_maps, plan


def kernel(input_data, connection_weights, connection_indices,
           delay_values, steps):
    global _compiled, _compiled_key
    from concourse.bass_utils import run_bass_kernel_spmd

    in_maps, plan = _preprocess(input_data, connection_weights,
                                connection_indices, delay_values, int(steps))
    key = repr(plan[0])
    if _compiled is None or _compiled_key != key:
        _compiled = _build_program(plan)
        _compiled_key = key
    res = run_bass_kernel_spmd(_compiled, in_maps, list(range(N_CORES)))

    pi, _ = _spatial_rank(connection_indices, delay_values)
    out_rank = np.zeros((BATCH, N_NEURONS), np.float32)
    for k in range(N_CORES):
        o = res.results[k]["out"]
        t0 = k * TGT_PER_CORE
        for tci in range(TCH):
            for r in range(BATCH):
                out_rank[r, t0 + tci * 128: t0 + (tci + 1) * 128] = \
                    o[:, tci * BATCH + r]
    out = np.zeros((BATCH, N_NEURONS), np.float32)
    out[:, pi] = out_rank
    return out[:, -INPUT_SIZE:].astype(np.float32)
